# revision 53
# baseline (speedup 1.0000x reference)
"""Sliding-window GQA attention on 8 TRN2 NeuronCores, tensor-parallel by heads.

Core c owns KV head c and Q heads 4c..4c+3.  All device matmuls run in bf16.
Structure: 4 sequence chunks of 512; per chunk QKV projection + RoPE, then
windowed attention (scores transposed [k,q], exp on ACT, post-exp 0/1 masks on
DVE), pv with a ones-column denominator, per-partition normalize, PE transpose
to [dh,s], then the wo out-projection.  Chunks pipeline: attention of chunk i
overlaps QKV of chunk i+1 on complementary engines.  Each core emits a partial
output (wo input-dim sharded); the host sums the 8 partials.

Perf notes vs the first working version (393.5us -> ~378us):
- PE warm-up matmuls at t=0 flip the HAM clock gate to 2.4 GHz early;
  exp's ACT spline table is preloaded during the DMA wait.
- Out-projection PSUM->SBUF staging copies moved off ACT (which the exp
  chain saturates) onto DVE (3/4) + ACT (1/4); post-exp mask multiplies
  moved DVE->GpSimd; chain PSUM copies moved DVE->ACT (idle then).
- x chunk tiles double-buffered so chunk i+1's DMA streams during chunk
  i's attention (paid for by halving the out staging tiles, streaming
  cos/sin per chunk, and single-buffering chain staging); w{q,k,v}
  pre-swizzled on the host so every weight DMA is fully contiguous;
  bulk DMAs not needed early (wo, chunk-1 x) issue from scalar-queue
  hook points so they don't dilute startup bandwidth.
- Chunk 0 runs all five projection chains grouped one x-group at a time
  (5 live PSUM accumulators -> 5x PE work per arriving group, no HAM
  re-throttle), and chunk 1's chains ride inside chunk 0's attention as
  the PE filler that out-projections provide for later chunks.
- The final chunk's last head pipelines transpose + out-projection per
  128-row block (no bare drain), and the tail stages out in 1024-col
  DMA pieces so the kernel doesn't end on one big exposed DMA.
"""

import os
import sys

sys.path.insert(0, "/opt/trn_rl_repo")

import numpy as np
import ml_dtypes

SEQ = 2048
DIM = 4096
N_HEADS = 32
N_KV = 8
HD = 128
WIN = 1024
NCORES = 8
QH = N_HEADS // N_KV          # 4 q heads per core
DHL = QH * HD                 # 512 local q dims
P = 128
DB = DIM // P                 # 32 contraction blocks
SC = 512                      # seq chunk
NSC = SEQ // SC               # 4 chunks
BPC = SC // P                 # 4 i-blocks per chunk
NIB = SEQ // P                # 16 blocks total
WB = WIN // P                 # 8 window blocks

BF = ml_dtypes.bfloat16


def _build_nc():
    import concourse.mybir as mybir
    from concourse import bacc
    from concourse.tile import TileContext

    f32 = mybir.dt.float32
    bf = mybir.dt.bfloat16

    nc = bacc.Bacc()
    xt = nc.declare_dram_parameter("xt", [DIM, SEQ], bf, isOutput=False)
    # w{q,k,v} are pre-swizzled on the host into the exact SBUF layout
    # ([partition, head, contraction-block, col] row-major) so every DMA is
    # a fully contiguous per-partition read.
    wqt = nc.declare_dram_parameter("wqt", [P, QH * DB * HD], bf,
                                    isOutput=False)
    wkt = nc.declare_dram_parameter("wkt", [P, DB * HD], bf, isOutput=False)
    wvt = nc.declare_dram_parameter("wvt", [P, DB * HD], bf, isOutput=False)
    wot = nc.declare_dram_parameter("wot", [DHL, DIM], bf, isOutput=False)
    cexp = nc.declare_dram_parameter("cexp", [P, SEQ], bf, isOutput=False)
    sexp = nc.declare_dram_parameter("sexp", [P, SEQ], bf, isOutput=False)
    rt = nc.declare_dram_parameter("rt", [P, P], bf, isOutput=False)
    ident = nc.declare_dram_parameter("ident", [P, P], bf, isOutput=False)
    md01 = nc.declare_dram_parameter("md01", [P, P], bf, isOutput=False)
    mt01 = nc.declare_dram_parameter("mt01", [P, P], bf, isOutput=False)
    out = nc.declare_dram_parameter("out", [SEQ, DIM], bf, isOutput=True)

    Exp = mybir.ActivationFunctionType.Exp

    with TileContext(nc) as tc:
        with (
            tc.tile_pool(name="const", bufs=1) as cp,
            tc.tile_pool(name="wp", bufs=1) as wp,
            tc.tile_pool(name="kvp", bufs=1) as kvp,
            tc.tile_pool(name="cs", bufs=2) as csp,
            tc.tile_pool(name="xtp", bufs=2) as xtp,
            tc.tile_pool(name="qrp", bufs=2) as qrp,
            tc.tile_pool(name="atp", bufs=2) as atp,
            tc.tile_pool(name="t12", bufs=2) as t12p,
            tc.tile_pool(name="expt", bufs=13) as etp,
            tc.tile_pool(name="asp", bufs=5) as asp,
            tc.tile_pool(name="osb", bufs=2) as osbp,
            tc.tile_pool(name="psb", bufs=2, space="PSUM") as psbig,
            tc.tile_pool(name="pss", bufs=2, space="PSUM") as pssc,
            tc.tile_pool(name="pvt", bufs=2, space="PSUM") as pspv,
            tc.tile_pool(name="pso", bufs=2, space="PSUM") as psop,
        ):
            xt_r = xt.rearrange("(o p) s -> p o s", p=P)

            # ---- PE warm-up: the HAM clock gate releases (1.2 -> 2.4 GHz)
            # only after a full free-running ~3.4us activity window is busy.
            # Burn ~6us on a throwaway accumulation while DMAs stream in.
            warm_src = cp.tile([P, 64], bf)
            nc.gpsimd.memset(warm_src[:], 0.25)
            # preload the exp spline table (~1.3us) off the critical path:
            # the first real exp otherwise pays it mid-attention
            warm_e = cp.tile([P, 1], bf)
            nc.scalar.activation(warm_e[:], warm_src[:, 0:1], Exp)
            pwarm = pssc.tile([P, SC], f32, tag="sc", name="pwarm")
            NWARM = 130
            for i in range(NWARM):
                nc.tensor.matmul(pwarm[0:64, 0:64], warm_src[:], warm_src[:],
                                 start=(i == 0), stop=(i == NWARM - 1))

            # ---- initial DMAs on two queues issuing in parallel: sync
            # carries wk + the x stream + wv; scalar carries the per-head
            # wq halves (low halves of all heads first -- the grouped
            # chains sweep db low-to-high across all heads).  Bulk data not
            # needed until later (wo, chunk-1 x) is issued from hook points
            # between the chain PSUM copies so it doesn't dilute startup
            # bandwidth.  w{k,q,v} sources are pre-swizzled: contiguous.
            wkt_r = wkt.rearrange("p (o m) -> p o m", m=HD)
            wqt_r = wqt.rearrange("p (h o m) -> p h o m", h=QH, m=HD)
            wvt_r = wvt.rearrange("p (o m) -> p o m", m=HD)
            wk_sb = wp.tile([P, DB, HD], bf)
            xs0 = xtp.tile([P, DB, SC], bf, tag="xs", name="xs0")
            wq_sb = wp.tile([P, QH, DB, HD], bf)
            wv_sb = wp.tile([P, DB, HD], bf)
            wo_sb = wp.tile([P, QH, DIM], bf)
            nc.sync.dma_start(wk_sb[:, 0:16, :], wkt_r[:, 0:16, :])
            nc.sync.dma_start(xs0[:, 0:4, :], xt_r[:, 0:4, 0:SC])
            nc.sync.dma_start(wk_sb[:, 16:32, :], wkt_r[:, 16:32, :])
            # wv early: chunk 0's phase-1 (k chain + v projection) is the
            # wq-independent PE work that bridges until wq lands
            nc.sync.dma_start(wv_sb[:], wvt_r[:])
            for g in range(1, 8):
                nc.sync.dma_start(xs0[:, g * 4:(g + 1) * 4, :],
                                  xt_r[:, g * 4:(g + 1) * 4, 0:SC])
            rt_sb = cp.tile([P, P], bf)
            nc.scalar.dma_start(rt_sb[:], rt[:])
            ce0 = csp.tile([P, SC], bf, tag="ce", name="ce0")
            nc.scalar.dma_start(ce0[:], cexp[:, 0:SC])
            se0 = csp.tile([P, SC], bf, tag="se", name="se0")
            nc.scalar.dma_start(se0[:], sexp[:, 0:SC])
            for hq in range(QH):
                nc.scalar.dma_start(wq_sb[:, hq], wqt_r[:, hq])
            id_sb = cp.tile([P, P], bf)
            nc.sync.dma_start(id_sb[:], ident[:])
            md_sb = cp.tile([P, P], bf)
            nc.sync.dma_start(md_sb[:], md01[:])
            mt_sb = cp.tile([P, P], bf)
            nc.sync.dma_start(mt_sb[:], mt01[:])

            krot = kvp.tile([P, SEQ], bf)          # kT rope'd [dh, s]
            v_sb = kvp.tile([P, NIB, HD + 1], bf)  # v natural [s, dh] + ones
            nc.gpsimd.memset(v_sb[:, :, HD:], 1.0)

            def emit_outproj(at, ci_src, sb, fine=False):
                # fine=True stages through per-oc tiles with one DMA each so
                # the kernel tail doesn't end on one big exposed DMA.
                r0 = ci_src * SC + sb * P
                for half in range(2):
                    ot = osbp.tile([P, DIM // 2], bf, tag="ot")
                    for hc in range(4):
                        oc = half * 4 + hc
                        po = psop.tile([P, 512], f32, tag="po")
                        for h2 in range(QH):
                            nc.tensor.matmul(
                                po, at[:, h2, sb * P:(sb + 1) * P],
                                wo_sb[:, h2, oc * 512:(oc + 1) * 512],
                                start=(h2 == 0), stop=(h2 == QH - 1))
                        # GpSimd cannot read PSUM; spread the staging copies
                        # over DVE (3/4) and ACT (1/4) instead.
                        if oc % 4 == 3:
                            nc.scalar.copy(ot[:, hc * 512:(hc + 1) * 512], po)
                        else:
                            nc.vector.tensor_copy(
                                ot[:, hc * 512:(hc + 1) * 512], po)
                        if fine and hc % 2 == 1:
                            # tail mode: stream the staged columns out in
                            # 1024-col pieces (issued as their copies land)
                            # so the kernel doesn't end on one big exposed
                            # DMA.  Sync queue only: the final drain barrier
                            # is only guaranteed to cover sync-issued DMAs.
                            c0 = half * (DIM // 2) + (hc - 1) * 512
                            nc.sync.dma_start(out[r0:r0 + P, c0:c0 + 1024],
                                              ot[:, (hc - 1) * 512:(hc + 1) * 512])
                    if not fine:
                        # alternate output DMAs over the sync and scalar
                        # queue rings: strided DRAM writes run ~90 GB/s per
                        # ring, and one ring backlogs into the kernel tail.
                        # Safe for every tile that gets reused later (the
                        # pool's WAR rotation forces DMA completion); only
                        # the final block's DMAs must be sync-issued.
                        eng = nc.scalar if half == 0 else nc.sync
                        eng.dma_start(
                            out[r0:r0 + P, half * (DIM // 2):(half + 1) * (DIM // 2)],
                            ot[:])

            def chain_w(hb, db):
                return (wk_sb[:, db, :] if hb == 0
                        else wq_sb[:, hb - 1, db, :])

            def emit_chain(hb, xs):
                ps = psbig.tile([P, SC], f32, tag="big", name="ps")
                for db in range(DB):
                    nc.tensor.matmul(ps, chain_w(hb, db), xs[:, db, :],
                                     start=(db == 0), stop=(db == DB - 1))
                qsb = t12p.tile([P, SC], bf, tag=f"qsb{hb}", bufs=1)
                nc.scalar.copy(qsb[:], ps)
                return qsb

            def emit_rope(hb, qsb, ce_sb, se_sb, krot_dst, qrot):
                # alternate pools: a 4-deep pr rotation so consecutive
                # rotate-matmuls never wait on the t2 multiply
                prp = psbig if hb % 2 == 0 else psop
                pr = prp.tile([P, SC], f32,
                              tag="big" if hb % 2 == 0 else "po",
                              name="pr")
                nc.tensor.matmul(pr, rt_sb[:], qsb[:], start=True, stop=True)
                t2 = t12p.tile([P, SC], bf, tag="t2")
                nc.vector.tensor_mul(t2[:], pr, se_sb[:])
                t1 = t12p.tile([P, SC], bf, tag="t1")
                nc.vector.tensor_mul(t1[:], qsb[:], ce_sb[:])
                dst = krot_dst if hb == 0 else qrot[:, hb - 1, :]
                nc.vector.tensor_add(dst, t1[:], t2[:])

            def emit_vblock(xs, bi0, sb2):
                pv2 = pssc.tile([P, HD], f32, tag="sc", name="pv2")
                for db in range(DB):
                    nc.tensor.matmul(pv2, xs[:, db, sb2 * P:(sb2 + 1) * P],
                                     wv_sb[:, db, :],
                                     start=(db == 0), stop=(db == DB - 1))
                nc.scalar.copy(v_sb[:, bi0 + sb2, :HD], pv2)

            def emit_scores(h, bi0, qrot):
                et = {}
                for bj in range(max(0, bi0 - WB), bi0 + BPC):
                    lo = max(bi0, bj)
                    hi = min(bi0 + BPC - 1, bj + WB)
                    qo0 = (lo - bi0) * P
                    w = (hi - lo + 1) * P
                    sc = pssc.tile([P, SC], f32, tag="sc", name="sc")
                    nc.tensor.matmul(sc[:, :w], krot[:, bj * P:(bj + 1) * P],
                                     qrot[:, h, qo0:qo0 + w],
                                     start=True, stop=True)
                    e = etp.tile([P, SC], bf, tag="et")
                    nc.scalar.activation(e[:, :w], sc[:, :w], Exp)
                    if bj >= bi0:  # diagonal block: causal upper-tri zero
                        io = (bj - lo) * P
                        nc.gpsimd.tensor_mul(e[:, io:io + P],
                                             e[:, io:io + P], md_sb[:])
                    if bj + WB <= bi0 + BPC - 1:  # tail block of window
                        io = (bj + WB - lo) * P
                        nc.gpsimd.tensor_mul(e[:, io:io + P],
                                             e[:, io:io + P], mt_sb[:])
                    et[bj] = (e, lo)
                return et

            def emit_pv_head(h, bi0, et, attnT, inline_out=None):
                # pv + normalize for all four blocks first; transposes
                # batched at the end so they never wait on the normalize.
                # inline_out=(ci) pipelines transpose+outproj per block so
                # the final chunk has no bare out-projection drain.
                asbs = []
                for bi in range(bi0, bi0 + BPC):
                    js = list(range(max(0, bi - WB), bi + 1))
                    pvp = pspv.tile([P, HD + 1], f32, tag="pvtr", name="pvp")
                    for idx, bj in enumerate(js):
                        e, lo = et[bj]
                        io = (bi - lo) * P
                        nc.tensor.matmul(pvp, e[:, io:io + P],
                                         v_sb[:, bj, :],
                                         start=(idx == 0),
                                         stop=(idx == len(js) - 1))
                    rec = asp.tile([P, 1], f32, tag="rec")
                    nc.vector.reciprocal(rec[:], pvp[:, HD:HD + 1])
                    asb = asp.tile([P, HD], bf, tag="asb")
                    nc.vector.tensor_scalar_mul(asb[:], pvp[:, :HD], rec[:])
                    asbs.append(asb)
                    if inline_out is not None:
                        k = bi - bi0
                        pt = pspv.tile([P, P], bf, tag="pvtr", name="pt")
                        nc.tensor.transpose(pt[:], asb[:], id_sb[:])
                        nc.vector.tensor_copy(attnT[:, h, k * P:(k + 1) * P],
                                              pt[:])
                        emit_outproj(attnT, inline_out, k, fine=(k == 3))
                if inline_out is None:
                    for k, bi in enumerate(range(bi0, bi0 + BPC)):
                        pt = pspv.tile([P, P], bf, tag="pvtr", name="pt")
                        nc.tensor.transpose(pt[:], asbs[k][:], id_sb[:])
                        nc.vector.tensor_copy(attnT[:, h, k * P:(k + 1) * P],
                                              pt[:])

            def emit_prefetch(ci):
                if ci + 1 >= NSC:
                    return None
                sn = (ci + 1) * SC
                nxt_xs = xtp.tile([P, DB, SC], bf, tag="xs")
                for g in range(4):
                    nc.sync.dma_start(nxt_xs[:, g * 8:(g + 1) * 8, :],
                                      xt_r[:, g * 8:(g + 1) * 8, sn:sn + SC])
                nxt_ce = csp.tile([P, SC], bf, tag="ce")
                nc.sync.dma_start(nxt_ce[:], cexp[:, sn:sn + SC])
                nxt_se = csp.tile([P, SC], bf, tag="se")
                nc.sync.dma_start(nxt_se[:], sexp[:, sn:sn + SC])
                return nxt_xs, nxt_ce, nxt_se

            # ---- chunk 0.  All five chains advance together one x-group
            # at a time (5 live PSUM accumulators), so the PE has 5x work
            # per arriving DMA group and stays busy (keeping the HAM clock
            # warm) while DMA-paced.  Chunk 0 has no out-projection filler,
            # so chunk 1's chains ride inside its attention to cover exp
            # latency.
            qrot = qrp.tile([P, QH, SC], bf, tag="qrot")
            attnT = atp.tile([P, QH, SC], bf, tag="attnT")
            # phase 1: k chain + v projection -- everything that does NOT
            # need wq -- advances one x-group at a time (5 live PSUM
            # accumulators, ~2x PE work per arriving group), bridging until
            # the wq heads finish streaming on the scalar ring
            ps_k = psbig.tile([P, SC], f32, tag="big", name="c0psk")
            psv = [psbig.tile([P, HD], f32, tag="big", name="c0psv0"),
                   psop.tile([P, HD], f32, tag="po", name="c0psv1"),
                   psop.tile([P, HD], f32, tag="po", name="c0psv2"),
                   pssc.tile([P, HD], f32, tag="sc", name="c0psv3")]
            for g in range(8):
                for db in range(g * 4, (g + 1) * 4):
                    nc.tensor.matmul(ps_k, chain_w(0, db), xs0[:, db, :],
                                     start=(db == 0), stop=(db == DB - 1))
                for sb2 in range(BPC):
                    for db in range(g * 4, (g + 1) * 4):
                        nc.tensor.matmul(
                            psv[sb2], xs0[:, db, sb2 * P:(sb2 + 1) * P],
                            wv_sb[:, db, :],
                            start=(db == 0), stop=(db == DB - 1))
            qsb_k = t12p.tile([P, SC], bf, tag="qsb0", bufs=1)
            nc.scalar.copy(qsb_k[:], ps_k)
            qsbs0 = [qsb_k]
            # deferred: wo is not needed until chunk 1's attention;
            # issuing it here keeps its 4 MB out of the startup burst
            nc.scalar.dma_start(
                wo_sb[:], wot.rearrange("(o p) m -> p o m", p=P))
            for sb2 in range(BPC):
                nc.scalar.copy(v_sb[:, sb2, :HD], psv[sb2])
            # phase 2: the four q chains, grouped (wq has landed by now)
            ps4 = [psbig.tile([P, SC], f32, tag="big", name=f"c0ps{i}")
                   for i in range(2)]
            ps4 += [psop.tile([P, SC], f32, tag="po", name=f"c0ps{i + 2}")
                    for i in range(2)]
            for g in range(8):
                for hb in range(1, QH + 1):
                    for db in range(g * 4, (g + 1) * 4):
                        nc.tensor.matmul(ps4[hb - 1], chain_w(hb, db),
                                         xs0[:, db, :],
                                         start=(db == 0), stop=(db == DB - 1))
            for hb in range(1, QH + 1):
                qsb = t12p.tile([P, SC], bf, tag=f"qsb{hb}", bufs=1)
                nc.scalar.copy(qsb[:], ps4[hb - 1])
                qsbs0.append(qsb)
            for hb in range(QH + 1):
                emit_rope(hb, qsbs0[hb], ce0, se0, krot[:, 0:SC], qrot)
            nxt = emit_prefetch(0)
            # chunk-0 attention with chunk-1 chains as the PE filler
            qsbs1 = []
            for h in range(QH):
                et = emit_scores(h, 0, qrot)
                qsbs1.append(emit_chain(h, nxt[0]))
                emit_pv_head(h, 0, et, attnT)
            prev = (attnT, 0)

            # ---- chunks 1..NSC-1
            for ci in range(1, NSC):
                s0 = ci * SC
                bi0 = ci * BPC
                xs, ce_sb, se_sb = nxt
                qrot = qrp.tile([P, QH, SC], bf, tag="qrot")

                # QKV projection chains (k first), then rope, then v.
                # For chunk 1 the first four chains already ran inside
                # chunk 0's attention.
                if ci == 1:
                    qsbs = qsbs1 + [emit_chain(QH, xs)]
                else:
                    qsbs = [emit_chain(hb, xs) for hb in range(QH + 1)]
                for hb in range(QH + 1):
                    emit_rope(hb, qsbs[hb], ce_sb, se_sb,
                              krot[:, s0:s0 + SC], qrot)
                for sb2 in range(BPC):
                    emit_vblock(xs, bi0, sb2)
                nxt = emit_prefetch(ci)

                # attention; previous chunk's out-projection rides along as
                # PE filler under each head's exp latency
                attnT = atp.tile([P, QH, SC], bf, tag="attnT")
                last = (ci == NSC - 1)
                for h in range(QH):
                    et = emit_scores(h, bi0, qrot)
                    emit_outproj(prev[0], prev[1], h)
                    inline = ci if (last and h == QH - 1) else None
                    emit_pv_head(h, bi0, et, attnT, inline_out=inline)
                prev = (attnT, ci)
    if not nc.is_finalized():
        nc.finalize()
    return nc


def _prep_inputs(x, wq, wk, wv, wo, cos, sin):
    scale = HD ** -0.5
    xtb = np.ascontiguousarray(x.T).astype(BF)
    ce = np.repeat(cos.T, 2, axis=0).astype(BF)          # [128, SEQ]
    se = np.repeat(sin.T, 2, axis=0).astype(BF)
    rtm = np.zeros((P, P), np.float32)
    for i in range(P // 2):
        rtm[2 * i, 2 * i + 1] = 1.0
        rtm[2 * i + 1, 2 * i] = -1.0
    rtm = rtm.astype(BF)
    idm = np.eye(P, dtype=np.float32).astype(BF)
    pp, ff = np.arange(P)[:, None], np.arange(P)[None, :]
    md = (pp <= ff).astype(np.float32).astype(BF)   # diag: keep k <= q
    mt = (ff < pp).astype(np.float32).astype(BF)    # tail: keep q < k

    in_maps = []
    for c in range(NCORES):
        qs, ks = slice(c * DHL, (c + 1) * DHL), slice(c * HD, (c + 1) * HD)
        # pre-swizzle wq/wk/wv into per-partition-linear SBUF layout:
        # [p, (h) o m] with value = wT[o*P + p, h*HD + m]
        wqT = (wq[qs] * scale).T.astype(np.float32)
        wql = np.ascontiguousarray(
            wqT.reshape(DB, P, QH, HD).transpose(1, 2, 0, 3)
            .reshape(P, QH * DB * HD)).astype(BF)
        wkT = wk[ks].T.astype(np.float32)
        wkl = np.ascontiguousarray(
            wkT.reshape(DB, P, HD).transpose(1, 0, 2)
            .reshape(P, DB * HD)).astype(BF)
        wvT = wv[ks].T.astype(np.float32)
        wvl = np.ascontiguousarray(
            wvT.reshape(DB, P, HD).transpose(1, 0, 2)
            .reshape(P, DB * HD)).astype(BF)
        in_maps.append({
            "xt": xtb,
            "wqt": wql,
            "wkt": wkl,
            "wvt": wvl,
            "wot": np.ascontiguousarray(wo[:, qs].T).astype(BF),
            "cexp": ce, "sexp": se, "rt": rtm, "ident": idm,
            "md01": md, "mt01": mt,
        })
    return in_maps


_NC_CACHE = {}


def kernel(x, wq, wk, wv, wo, cos, sin):
    from concourse.bass_utils import run_bass_kernel_spmd

    x = np.asarray(x, np.float32)
    wq = np.asarray(wq, np.float32)
    wk = np.asarray(wk, np.float32)
    wv = np.asarray(wv, np.float32)
    wo = np.asarray(wo, np.float32)
    cos = np.asarray(cos, np.float32)
    sin = np.asarray(sin, np.float32)

    if "nc" not in _NC_CACHE:
        _NC_CACHE["nc"] = _build_nc()
    nc = _NC_CACHE["nc"]
    in_maps = _prep_inputs(x, wq, wk, wv, wo, cos, sin)

    trace = os.environ.get("KERNEL_TRACE", "0") == "1"
    res = None
    if trace:
        try:
            res = run_bass_kernel_spmd(nc, in_maps,
                                       core_ids=list(range(NCORES)),
                                       trace=True)
        except Exception as e:  # profiling hooks absent in some containers
            print(f"trace unavailable ({type(e).__name__}: {e}); "
                  "running untraced")
            res = None
    if res is None:
        res = run_bass_kernel_spmd(nc, in_maps, core_ids=list(range(NCORES)))
    if res.exec_time_ns is not None:
        print(f"HW exec time: {res.exec_time_ns} ns")
    acc = np.zeros((SEQ, DIM), np.float32)
    for c in range(NCORES):
        acc += res.results[c]["out"].astype(np.float32)
    return acc


# revision 54
# speedup vs baseline: 1.0439x; 1.0439x over previous
"""Sliding-window GQA attention on 8 TRN2 NeuronCores, tensor-parallel by heads.

Core c owns KV head c and Q heads 4c..4c+3.  All device matmuls run in bf16.
Structure: 4 sequence chunks of 512; per chunk QKV projection + RoPE, then
windowed attention (scores transposed [k,q], exp on ACT, post-exp 0/1 masks on
DVE), pv with a ones-column denominator, per-partition normalize, PE transpose
to [dh,s], then the wo out-projection.  Chunks pipeline: attention of chunk i
overlaps QKV of chunk i+1 on complementary engines.  Each core emits a partial
output (wo input-dim sharded); the host sums the 8 partials.

Perf notes vs the first working version (393.5us -> ~378us):
- PE warm-up matmuls at t=0 flip the HAM clock gate to 2.4 GHz early;
  exp's ACT spline table is preloaded during the DMA wait.
- Out-projection PSUM->SBUF staging copies moved off ACT (which the exp
  chain saturates) onto DVE (3/4) + ACT (1/4); post-exp mask multiplies
  moved DVE->GpSimd; chain PSUM copies moved DVE->ACT (idle then).
- x chunk tiles double-buffered so chunk i+1's DMA streams during chunk
  i's attention (paid for by halving the out staging tiles, streaming
  cos/sin per chunk, and single-buffering chain staging); w{q,k,v}
  pre-swizzled on the host so every weight DMA is fully contiguous;
  bulk DMAs not needed early (wo, chunk-1 x) issue from scalar-queue
  hook points so they don't dilute startup bandwidth.
- Chunk 0 runs all five projection chains grouped one x-group at a time
  (5 live PSUM accumulators -> 5x PE work per arriving group, no HAM
  re-throttle), and chunk 1's chains ride inside chunk 0's attention as
  the PE filler that out-projections provide for later chunks.
- The final chunk's last head pipelines transpose + out-projection per
  128-row block (no bare drain), and the tail stages out in 1024-col
  DMA pieces so the kernel doesn't end on one big exposed DMA.
"""

import os
import sys

sys.path.insert(0, "/opt/trn_rl_repo")

import numpy as np
import ml_dtypes

SEQ = 2048
DIM = 4096
N_HEADS = 32
N_KV = 8
HD = 128
WIN = 1024
NCORES = 8
QH = N_HEADS // N_KV          # 4 q heads per core
DHL = QH * HD                 # 512 local q dims
P = 128
DB = DIM // P                 # 32 contraction blocks
SC = 512                      # seq chunk
NSC = SEQ // SC               # 4 chunks
BPC = SC // P                 # 4 i-blocks per chunk
NIB = SEQ // P                # 16 blocks total
WB = WIN // P                 # 8 window blocks

BF = ml_dtypes.bfloat16


def _build_nc():
    import concourse.mybir as mybir
    from concourse import bacc
    from concourse.tile import TileContext

    f32 = mybir.dt.float32
    bf = mybir.dt.bfloat16

    nc = bacc.Bacc()
    xt = nc.declare_dram_parameter("xt", [DIM, SEQ], bf, isOutput=False)
    # w{q,k,v} are pre-swizzled on the host into the exact SBUF layout
    # ([partition, head, contraction-block, col] row-major) so every DMA is
    # a fully contiguous per-partition read.
    wqt = nc.declare_dram_parameter("wqt", [P, QH * DB * HD], bf,
                                    isOutput=False)
    wkt = nc.declare_dram_parameter("wkt", [P, DB * HD], bf, isOutput=False)
    wvt = nc.declare_dram_parameter("wvt", [P, DB * HD], bf, isOutput=False)
    wot = nc.declare_dram_parameter("wot", [DHL, DIM], bf, isOutput=False)
    cexp = nc.declare_dram_parameter("cexp", [P, SEQ], bf, isOutput=False)
    sexp = nc.declare_dram_parameter("sexp", [P, SEQ], bf, isOutput=False)
    rt = nc.declare_dram_parameter("rt", [P, P], bf, isOutput=False)
    ident = nc.declare_dram_parameter("ident", [P, P], bf, isOutput=False)
    md01 = nc.declare_dram_parameter("md01", [P, P], bf, isOutput=False)
    mt01 = nc.declare_dram_parameter("mt01", [P, P], bf, isOutput=False)
    out = nc.declare_dram_parameter("out", [SEQ, DIM], bf, isOutput=True)

    Exp = mybir.ActivationFunctionType.Exp

    with TileContext(nc) as tc:
        with (
            tc.tile_pool(name="const", bufs=1) as cp,
            tc.tile_pool(name="wp", bufs=1) as wp,
            tc.tile_pool(name="kvp", bufs=1) as kvp,
            tc.tile_pool(name="cs", bufs=2) as csp,
            tc.tile_pool(name="xtp", bufs=2) as xtp,
            tc.tile_pool(name="qrp", bufs=2) as qrp,
            tc.tile_pool(name="atp", bufs=2) as atp,
            tc.tile_pool(name="t12", bufs=2) as t12p,
            tc.tile_pool(name="expt", bufs=13) as etp,
            tc.tile_pool(name="asp", bufs=5) as asp,
            tc.tile_pool(name="osb", bufs=2) as osbp,
            tc.tile_pool(name="psb", bufs=2, space="PSUM") as psbig,
            tc.tile_pool(name="pss", bufs=2, space="PSUM") as pssc,
            tc.tile_pool(name="pvt", bufs=2, space="PSUM") as pspv,
            tc.tile_pool(name="pso", bufs=2, space="PSUM") as psop,
        ):
            xt_r = xt.rearrange("(o p) s -> p o s", p=P)

            # ---- PE warm-up: the HAM clock gate releases (1.2 -> 2.4 GHz)
            # only after a full free-running ~3.4us activity window is busy.
            # Burn ~6us on a throwaway accumulation while DMAs stream in.
            warm_src = cp.tile([P, 64], bf)
            nc.gpsimd.memset(warm_src[:], 0.25)
            # preload the exp spline table (~1.3us) off the critical path:
            # the first real exp otherwise pays it mid-attention
            warm_e = cp.tile([P, 1], bf)
            nc.scalar.activation(warm_e[:], warm_src[:, 0:1], Exp)
            pwarm = pssc.tile([P, SC], f32, tag="sc", name="pwarm")
            NWARM = 130
            for i in range(NWARM):
                nc.tensor.matmul(pwarm[0:64, 0:64], warm_src[:], warm_src[:],
                                 start=(i == 0), stop=(i == NWARM - 1))

            # ---- initial DMAs on two queues issuing in parallel: sync
            # carries wk + the x stream + wv; scalar carries the per-head
            # wq halves (low halves of all heads first -- the grouped
            # chains sweep db low-to-high across all heads).  Bulk data not
            # needed until later (wo, chunk-1 x) is issued from hook points
            # between the chain PSUM copies so it doesn't dilute startup
            # bandwidth.  w{k,q,v} sources are pre-swizzled: contiguous.
            wkt_r = wkt.rearrange("p (o m) -> p o m", m=HD)
            wqt_r = wqt.rearrange("p (h o m) -> p h o m", h=QH, m=HD)
            wvt_r = wvt.rearrange("p (o m) -> p o m", m=HD)
            wk_sb = wp.tile([P, DB, HD], bf)
            xs0 = xtp.tile([P, DB, SC], bf, tag="xs", name="xs0")
            wq_sb = wp.tile([P, QH, DB, HD], bf)
            wv_sb = wp.tile([P, DB, HD], bf)
            wo_sb = wp.tile([P, QH, DIM], bf)
            nc.sync.dma_start(wk_sb[:, 0:16, :], wkt_r[:, 0:16, :])
            nc.sync.dma_start(xs0[:, 0:4, :], xt_r[:, 0:4, 0:SC])
            nc.sync.dma_start(wk_sb[:, 16:32, :], wkt_r[:, 16:32, :])
            for g in range(1, 8):
                nc.sync.dma_start(xs0[:, g * 4:(g + 1) * 4, :],
                                  xt_r[:, g * 4:(g + 1) * 4, 0:SC])
            rt_sb = cp.tile([P, P], bf)
            nc.scalar.dma_start(rt_sb[:], rt[:])
            ce0 = csp.tile([P, SC], bf, tag="ce", name="ce0")
            nc.scalar.dma_start(ce0[:], cexp[:, 0:SC])
            se0 = csp.tile([P, SC], bf, tag="se", name="se0")
            nc.scalar.dma_start(se0[:], sexp[:, 0:SC])
            for hq in range(QH):
                nc.scalar.dma_start(wq_sb[:, hq], wqt_r[:, hq])
            nc.sync.dma_start(wv_sb[:], wvt_r[:])
            id_sb = cp.tile([P, P], bf)
            nc.sync.dma_start(id_sb[:], ident[:])
            md_sb = cp.tile([P, P], bf)
            nc.sync.dma_start(md_sb[:], md01[:])
            mt_sb = cp.tile([P, P], bf)
            nc.sync.dma_start(mt_sb[:], mt01[:])

            krot = kvp.tile([P, SEQ], bf)          # kT rope'd [dh, s]
            v_sb = kvp.tile([P, NIB, HD + 1], bf)  # v natural [s, dh] + ones
            nc.gpsimd.memset(v_sb[:, :, HD:], 1.0)

            def emit_outproj(at, ci_src, sb, fine=False):
                # fine=True stages through per-oc tiles with one DMA each so
                # the kernel tail doesn't end on one big exposed DMA.
                r0 = ci_src * SC + sb * P
                for half in range(2):
                    ot = osbp.tile([P, DIM // 2], bf, tag="ot")
                    for hc in range(4):
                        oc = half * 4 + hc
                        po = psop.tile([P, 512], f32, tag="po")
                        for h2 in range(QH):
                            nc.tensor.matmul(
                                po, at[:, h2, sb * P:(sb + 1) * P],
                                wo_sb[:, h2, oc * 512:(oc + 1) * 512],
                                start=(h2 == 0), stop=(h2 == QH - 1))
                        # GpSimd cannot read PSUM; spread the staging copies
                        # over DVE (3/4) and ACT (1/4) instead.
                        if oc % 4 == 3:
                            nc.scalar.copy(ot[:, hc * 512:(hc + 1) * 512], po)
                        else:
                            nc.vector.tensor_copy(
                                ot[:, hc * 512:(hc + 1) * 512], po)
                        if fine and hc % 2 == 1:
                            # tail mode: stream the staged columns out in
                            # 1024-col pieces (issued as their copies land)
                            # so the kernel doesn't end on one big exposed
                            # DMA.  Sync queue only: the final drain barrier
                            # is only guaranteed to cover sync-issued DMAs.
                            c0 = half * (DIM // 2) + (hc - 1) * 512
                            nc.sync.dma_start(out[r0:r0 + P, c0:c0 + 1024],
                                              ot[:, (hc - 1) * 512:(hc + 1) * 512])
                    if not fine:
                        # alternate output DMAs over the sync and scalar
                        # queue rings: strided DRAM writes run ~90 GB/s per
                        # ring, and one ring backlogs into the kernel tail.
                        # Safe for every tile that gets reused later (the
                        # pool's WAR rotation forces DMA completion); only
                        # the final block's DMAs must be sync-issued.
                        eng = nc.scalar if half == 0 else nc.sync
                        eng.dma_start(
                            out[r0:r0 + P, half * (DIM // 2):(half + 1) * (DIM // 2)],
                            ot[:])

            def chain_w(hb, db):
                return (wk_sb[:, db, :] if hb == 0
                        else wq_sb[:, hb - 1, db, :])

            def emit_chain(hb, xs):
                ps = psbig.tile([P, SC], f32, tag="big", name="ps")
                for db in range(DB):
                    nc.tensor.matmul(ps, chain_w(hb, db), xs[:, db, :],
                                     start=(db == 0), stop=(db == DB - 1))
                qsb = t12p.tile([P, SC], bf, tag=f"qsb{hb}", bufs=1)
                nc.scalar.copy(qsb[:], ps)
                return qsb

            def emit_rope(hb, qsb, ce_sb, se_sb, krot_dst, qrot):
                # alternate pools: a 4-deep pr rotation so consecutive
                # rotate-matmuls never wait on the t2 multiply
                prp = psbig if hb % 2 == 0 else psop
                pr = prp.tile([P, SC], f32,
                              tag="big" if hb % 2 == 0 else "po",
                              name="pr")
                nc.tensor.matmul(pr, rt_sb[:], qsb[:], start=True, stop=True)
                t2 = t12p.tile([P, SC], bf, tag="t2")
                nc.vector.tensor_mul(t2[:], pr, se_sb[:])
                t1 = t12p.tile([P, SC], bf, tag="t1")
                nc.vector.tensor_mul(t1[:], qsb[:], ce_sb[:])
                dst = krot_dst if hb == 0 else qrot[:, hb - 1, :]
                nc.vector.tensor_add(dst, t1[:], t2[:])

            def emit_vblock(xs, bi0, sb2):
                pv2 = pssc.tile([P, HD], f32, tag="sc", name="pv2")
                for db in range(DB):
                    nc.tensor.matmul(pv2, xs[:, db, sb2 * P:(sb2 + 1) * P],
                                     wv_sb[:, db, :],
                                     start=(db == 0), stop=(db == DB - 1))
                nc.scalar.copy(v_sb[:, bi0 + sb2, :HD], pv2)

            def emit_scores(h, bi0, qrot):
                et = {}
                for bj in range(max(0, bi0 - WB), bi0 + BPC):
                    lo = max(bi0, bj)
                    hi = min(bi0 + BPC - 1, bj + WB)
                    qo0 = (lo - bi0) * P
                    w = (hi - lo + 1) * P
                    sc = pssc.tile([P, SC], f32, tag="sc", name="sc")
                    nc.tensor.matmul(sc[:, :w], krot[:, bj * P:(bj + 1) * P],
                                     qrot[:, h, qo0:qo0 + w],
                                     start=True, stop=True)
                    e = etp.tile([P, SC], bf, tag="et")
                    nc.scalar.activation(e[:, :w], sc[:, :w], Exp)
                    if bj >= bi0:  # diagonal block: causal upper-tri zero
                        io = (bj - lo) * P
                        nc.gpsimd.tensor_mul(e[:, io:io + P],
                                             e[:, io:io + P], md_sb[:])
                    if bj + WB <= bi0 + BPC - 1:  # tail block of window
                        io = (bj + WB - lo) * P
                        nc.gpsimd.tensor_mul(e[:, io:io + P],
                                             e[:, io:io + P], mt_sb[:])
                    et[bj] = (e, lo)
                return et

            def emit_pv_head(h, bi0, et, attnT, inline_out=None):
                # pv + normalize for all four blocks first; transposes
                # batched at the end so they never wait on the normalize.
                # inline_out=(ci) pipelines transpose+outproj per block so
                # the final chunk has no bare out-projection drain.
                asbs = []
                for bi in range(bi0, bi0 + BPC):
                    js = list(range(max(0, bi - WB), bi + 1))
                    pvp = pspv.tile([P, HD + 1], f32, tag="pvtr", name="pvp")
                    for idx, bj in enumerate(js):
                        e, lo = et[bj]
                        io = (bi - lo) * P
                        nc.tensor.matmul(pvp, e[:, io:io + P],
                                         v_sb[:, bj, :],
                                         start=(idx == 0),
                                         stop=(idx == len(js) - 1))
                    rec = asp.tile([P, 1], f32, tag="rec")
                    nc.vector.reciprocal(rec[:], pvp[:, HD:HD + 1])
                    asb = asp.tile([P, HD], bf, tag="asb")
                    nc.vector.tensor_scalar_mul(asb[:], pvp[:, :HD], rec[:])
                    asbs.append(asb)
                    if inline_out is not None:
                        k = bi - bi0
                        pt = pspv.tile([P, P], bf, tag="pvtr", name="pt")
                        nc.tensor.transpose(pt[:], asb[:], id_sb[:])
                        nc.vector.tensor_copy(attnT[:, h, k * P:(k + 1) * P],
                                              pt[:])
                        emit_outproj(attnT, inline_out, k, fine=(k == 3))
                if inline_out is None:
                    for k, bi in enumerate(range(bi0, bi0 + BPC)):
                        pt = pspv.tile([P, P], bf, tag="pvtr", name="pt")
                        nc.tensor.transpose(pt[:], asbs[k][:], id_sb[:])
                        nc.vector.tensor_copy(attnT[:, h, k * P:(k + 1) * P],
                                              pt[:])

            def emit_prefetch(ci):
                if ci + 1 >= NSC:
                    return None
                sn = (ci + 1) * SC
                nxt_xs = xtp.tile([P, DB, SC], bf, tag="xs")
                for g in range(4):
                    nc.sync.dma_start(nxt_xs[:, g * 8:(g + 1) * 8, :],
                                      xt_r[:, g * 8:(g + 1) * 8, sn:sn + SC])
                nxt_ce = csp.tile([P, SC], bf, tag="ce")
                nc.sync.dma_start(nxt_ce[:], cexp[:, sn:sn + SC])
                nxt_se = csp.tile([P, SC], bf, tag="se")
                nc.sync.dma_start(nxt_se[:], sexp[:, sn:sn + SC])
                return nxt_xs, nxt_ce, nxt_se

            # ---- chunk 0.  All five chains advance together one x-group
            # at a time (5 live PSUM accumulators), so the PE has 5x work
            # per arriving DMA group and stays busy (keeping the HAM clock
            # warm) while DMA-paced.  Chunk 0 has no out-projection filler,
            # so chunk 1's chains ride inside its attention to cover exp
            # latency.
            qrot = qrp.tile([P, QH, SC], bf, tag="qrot")
            attnT = atp.tile([P, QH, SC], bf, tag="attnT")
            ps5 = [psbig.tile([P, SC], f32, tag="big", name=f"c0ps{hb}")
                   for hb in range(2)]
            ps5 += [psop.tile([P, SC], f32, tag="po", name=f"c0ps{hb + 2}")
                    for hb in range(2)]
            ps5.append(pssc.tile([P, SC], f32, tag="sc", name="c0ps4"))
            for g in range(8):
                for hb in range(QH + 1):
                    for db in range(g * 4, (g + 1) * 4):
                        nc.tensor.matmul(ps5[hb], chain_w(hb, db),
                                         xs0[:, db, :],
                                         start=(db == 0), stop=(db == DB - 1))
            qsbs0 = []
            for hb in range(QH + 1):
                qsb = t12p.tile([P, SC], bf, tag=f"qsb{hb}", bufs=1)
                nc.scalar.copy(qsb[:], ps5[hb])
                qsbs0.append(qsb)
                if hb == 0:
                    # deferred: wo is not needed until chunk 1's attention;
                    # issuing it here keeps its 4 MB out of the startup burst
                    nc.scalar.dma_start(
                        wo_sb[:], wot.rearrange("(o p) m -> p o m", p=P))
            for hb in range(QH + 1):
                emit_rope(hb, qsbs0[hb], ce0, se0, krot[:, 0:SC], qrot)
            for sb2 in range(BPC):
                emit_vblock(xs0, 0, sb2)
            nxt = emit_prefetch(0)
            # chunk-0 attention with chunk-1 chains as the PE filler
            qsbs1 = []
            for h in range(QH):
                et = emit_scores(h, 0, qrot)
                qsbs1.append(emit_chain(h, nxt[0]))
                emit_pv_head(h, 0, et, attnT)
            prev = (attnT, 0)

            # ---- chunks 1..NSC-1
            for ci in range(1, NSC):
                s0 = ci * SC
                bi0 = ci * BPC
                xs, ce_sb, se_sb = nxt
                qrot = qrp.tile([P, QH, SC], bf, tag="qrot")

                # QKV projection chains (k first), then rope, then v.
                # For chunk 1 the first four chains already ran inside
                # chunk 0's attention.
                if ci == 1:
                    qsbs = qsbs1 + [emit_chain(QH, xs)]
                else:
                    qsbs = [emit_chain(hb, xs) for hb in range(QH + 1)]
                for hb in range(QH + 1):
                    emit_rope(hb, qsbs[hb], ce_sb, se_sb,
                              krot[:, s0:s0 + SC], qrot)
                for sb2 in range(BPC):
                    emit_vblock(xs, bi0, sb2)
                nxt = emit_prefetch(ci)

                # attention; previous chunk's out-projection rides along as
                # PE filler under each head's exp latency
                attnT = atp.tile([P, QH, SC], bf, tag="attnT")
                last = (ci == NSC - 1)
                for h in range(QH):
                    et = emit_scores(h, bi0, qrot)
                    emit_outproj(prev[0], prev[1], h)
                    inline = ci if (last and h == QH - 1) else None
                    emit_pv_head(h, bi0, et, attnT, inline_out=inline)
                prev = (attnT, ci)
    if not nc.is_finalized():
        nc.finalize()
    return nc


def _prep_inputs(x, wq, wk, wv, wo, cos, sin):
    scale = HD ** -0.5
    xtb = np.ascontiguousarray(x.T).astype(BF)
    ce = np.repeat(cos.T, 2, axis=0).astype(BF)          # [128, SEQ]
    se = np.repeat(sin.T, 2, axis=0).astype(BF)
    rtm = np.zeros((P, P), np.float32)
    for i in range(P // 2):
        rtm[2 * i, 2 * i + 1] = 1.0
        rtm[2 * i + 1, 2 * i] = -1.0
    rtm = rtm.astype(BF)
    idm = np.eye(P, dtype=np.float32).astype(BF)
    pp, ff = np.arange(P)[:, None], np.arange(P)[None, :]
    md = (pp <= ff).astype(np.float32).astype(BF)   # diag: keep k <= q
    mt = (ff < pp).astype(np.float32).astype(BF)    # tail: keep q < k

    in_maps = []
    for c in range(NCORES):
        qs, ks = slice(c * DHL, (c + 1) * DHL), slice(c * HD, (c + 1) * HD)
        # pre-swizzle wq/wk/wv into per-partition-linear SBUF layout:
        # [p, (h) o m] with value = wT[o*P + p, h*HD + m]
        wqT = (wq[qs] * scale).T.astype(np.float32)
        wql = np.ascontiguousarray(
            wqT.reshape(DB, P, QH, HD).transpose(1, 2, 0, 3)
            .reshape(P, QH * DB * HD)).astype(BF)
        wkT = wk[ks].T.astype(np.float32)
        wkl = np.ascontiguousarray(
            wkT.reshape(DB, P, HD).transpose(1, 0, 2)
            .reshape(P, DB * HD)).astype(BF)
        wvT = wv[ks].T.astype(np.float32)
        wvl = np.ascontiguousarray(
            wvT.reshape(DB, P, HD).transpose(1, 0, 2)
            .reshape(P, DB * HD)).astype(BF)
        in_maps.append({
            "xt": xtb,
            "wqt": wql,
            "wkt": wkl,
            "wvt": wvl,
            "wot": np.ascontiguousarray(wo[:, qs].T).astype(BF),
            "cexp": ce, "sexp": se, "rt": rtm, "ident": idm,
            "md01": md, "mt01": mt,
        })
    return in_maps


_NC_CACHE = {}


def kernel(x, wq, wk, wv, wo, cos, sin):
    from concourse.bass_utils import run_bass_kernel_spmd

    x = np.asarray(x, np.float32)
    wq = np.asarray(wq, np.float32)
    wk = np.asarray(wk, np.float32)
    wv = np.asarray(wv, np.float32)
    wo = np.asarray(wo, np.float32)
    cos = np.asarray(cos, np.float32)
    sin = np.asarray(sin, np.float32)

    if "nc" not in _NC_CACHE:
        _NC_CACHE["nc"] = _build_nc()
    nc = _NC_CACHE["nc"]
    in_maps = _prep_inputs(x, wq, wk, wv, wo, cos, sin)

    trace = os.environ.get("KERNEL_TRACE", "0") == "1"
    res = None
    if trace:
        try:
            res = run_bass_kernel_spmd(nc, in_maps,
                                       core_ids=list(range(NCORES)),
                                       trace=True)
        except Exception as e:  # profiling hooks absent in some containers
            print(f"trace unavailable ({type(e).__name__}: {e}); "
                  "running untraced")
            res = None
    if res is None:
        res = run_bass_kernel_spmd(nc, in_maps, core_ids=list(range(NCORES)))
    if res.exec_time_ns is not None:
        print(f"HW exec time: {res.exec_time_ns} ns")
    acc = np.zeros((SEQ, DIM), np.float32)
    for c in range(NCORES):
        acc += res.results[c]["out"].astype(np.float32)
    return acc
